# revision 40
# baseline (speedup 1.0000x reference)
"""AttentionPairBias kernel for 8 Trainium2 NeuronCores (v3, bf16 + cc).

Sharding: data-parallel over (batch, query-row-block). Core c handles batch
b = c // 4 and query rows i in [(c % 4) * 128, (c % 4 + 1) * 128).
Each core computes the full 16-head attention for its 128 query rows.

v3 changes vs v2 (298us):
  - k/v projections are column-sharded over the 4 cores of a batch (each
    core computes 4 heads' worth: kT dc-chunks {2*ib, 2*ib+1}, v head-cols
    [256*ib, 256*ib+256)), then AllGather'd within the 4-core replica group
    via DRAM bounce buffers on the gpsimd ring. Removes the 4x-redundant
    k/v compute (48k PE cycles) and 6MB of weight DMA per core.
  - k/v slice matmuls issue before the z loop; the collective overlaps the
    long z phase; read-back + remaining weight loads ride the gpsimd queue.
  - wq/wg/wo loads are interleaved into the z loop so they don't hog HBM
    bandwidth at t=0 (phase-1 z streaming starts immediately).
  - deeper zin prefetch (8 bufs).

v2 changes vs v1 (427us):
  - z loaded as a single bf16 plane; mean-centering folded into the
    stationary (u' = ln_g*wz - su/128) so phase 1 is 2 matmul passes per
    row; all weights bf16; zu roundtrip bf16 (plane 16 = musum, 17 =
    sumsq); attention in bf16 without softmax max-subtract; DMA spread
    over sync/scalar/gpsimd rings.
"""

import sys

sys.path.insert(0, "/opt/trn_rl_repo")

from contextlib import ExitStack

import numpy as np

import concourse.bacc as bacc
import concourse.bass as bass
import concourse.mybir as mybir
import concourse.tile as tile
from concourse.bass_utils import run_bass_kernel_spmd
from concourse.masks import make_identity

F32 = mybir.dt.float32
BF16 = mybir.dt.bfloat16
F8E4 = mybir.dt.float8e4
AF = mybir.ActivationFunctionType
ALU = mybir.AluOpType

B, N, CS, CZ, H, D = 2, 512, 1024, 128, 16, 64
ROWS = 128          # query rows per core
NCHUNK = CS // 128  # 8 contraction chunks of 128
N_CORES = 8
EPS = 1e-5
USE_CC = True       # AllGather k/v across the 4 cores of a batch

_CACHE = {}


def _build_program(mask_trivial: bool):
    nc = bacc.Bacc("TRN2", target_bir_lowering=False, debug=False,
                   num_devices=N_CORES)

    def din(name, shape):
        return nc.dram_tensor(name, shape, F32, kind="ExternalInput").ap()

    # bf16 tensors bit-packed into f32-typed dram tensors (axon PJRT path
    # prefers f32 jit parameters); bitcast to BF16 on-chip.
    sT_d = din("sT", (128, NCHUNK, ROWS // 2))
    kinT_d = din("kinT", (128, NCHUNK, N // 2))
    zh_d = din("zh", (CZ, ROWS, N // 2))
    wq_d = din("wq", (128, NCHUNK, CS // 2))
    if USE_CC:
        wk_d = din("wk", (128, NCHUNK, 128))   # 256-col slice, bf16-packed
        wv_d = din("wv", (128, NCHUNK, 128))
    else:
        wk_d = din("wk", (128, NCHUNK, CS // 2))
        wv_d = din("wv", (128, NCHUNK, CS // 2))
    wg_d = din("wg", (128, NCHUNK, CS // 2))
    wo_d = din("wo", (128, NCHUNK, CS // 2))
    bq_d = din("bqt", (128, NCHUNK))
    lng_d = din("lng", (CZ, 1))
    lnb_d = din("lnb", (CZ, 1))
    wz_d = din("wz", (CZ, H))
    if not mask_trivial:
        mneg_d = din("mneg", (1, N))
    out_d = nc.dram_tensor("out", (ROWS, CS), F32, kind="ExternalOutput").ap()

    with tile.TileContext(nc) as tc, ExitStack() as ctx:
        dram = ctx.enter_context(tc.tile_pool(name="dram", bufs=1, space="DRAM"))
        # planes 0..15: centered zu per head; 16: musum; 17: sumsq
        zu_d = dram.tile([18, ROWS, N], BF16)
        if USE_CC:
            kv_in = dram.tile([128, 2048], BF16, name="kv_in")
            kv_out = dram.tile([4, 128, 2048], BF16, name="kv_out")

        const = ctx.enter_context(tc.tile_pool(name="const", bufs=1))
        small = ctx.enter_context(tc.tile_pool(name="small", bufs=1))

        ident_bf = const.tile([128, 128], BF16)
        make_identity(nc, ident_bf[:])
        ones = const.tile([128, 128], F32)
        nc.vector.memset(ones[:], 1.0)

        wz_sb = small.tile([CZ, H], F32)
        nc.sync.dma_start(wz_sb[:], wz_d[:])
        lng_sb = small.tile([CZ, 1], F32)
        nc.sync.dma_start(lng_sb[:], lng_d[:])
        lnb_sb = small.tile([CZ, 1], F32)
        nc.sync.dma_start(lnb_sb[:], lnb_d[:])
        bq_sb = small.tile([128, NCHUNK], F32)
        nc.sync.dma_start(bq_sb[:], bq_d[:])

        u_f = small.tile([CZ, H], F32)
        nc.vector.tensor_tensor(u_f[:], wz_sb[:],
                                lng_sb[:, 0:1].to_broadcast([CZ, H]), ALU.mult)
        bwz = small.tile([CZ, H], F32)
        nc.vector.tensor_tensor(bwz[:], wz_sb[:],
                                lnb_sb[:, 0:1].to_broadcast([CZ, H]), ALU.mult)

        msu_b = small.tile([128, H], F32)   # -su[h]/128 replicated on partitions
        t_b = small.tile([128, H], F32)
        with ExitStack() as pctx:
            ppre = pctx.enter_context(tc.tile_pool(name="ppre", bufs=1,
                                                   space="PSUM"))
            su_ps = ppre.tile([128, H], F32, tag="pre")
            nc.tensor.matmul(su_ps[:], ones[:], u_f[:], start=True, stop=True)
            nc.vector.tensor_scalar_mul(msu_b[:], su_ps[:], -1.0 / CZ)
            t_ps = ppre.tile([128, H], F32, tag="pre")
            nc.tensor.matmul(t_ps[:], ones[:], bwz[:], start=True, stop=True)
            nc.vector.tensor_copy(t_b[:], t_ps[:])
        # u' = u - su/128 (mean-centering folded into the stationary), bf16,
        # packed [u'(16) | ones(1) | zeros..] in a 32-wide stationary.
        uc_f = small.tile([CZ, H], F32)
        nc.vector.tensor_tensor(uc_f[:], u_f[:], msu_b[0:CZ, :], ALU.add)
        u_bf = const.tile([CZ, 32], BF16)
        nc.vector.memset(u_bf[:], 0.0)
        nc.vector.tensor_copy(u_bf[:, 0:H], uc_f[:])
        nc.vector.memset(u_bf[:, H:H + 1], 1.0)
        # sq-pass stationary for fp8 DoubleRow: k-tile 0 selects output row 0,
        # k-tile 1 selects row 1, so one 256-col stream yields the per-column
        # sums of both j-halves of z^2 at 0.5 cycles/col
        u_sq = const.tile([CZ, 2, 32], F8E4)
        nc.vector.memset(u_sq[:], 0.0)
        nc.vector.memset(u_sq[:, 0, 0:1], 1.0)
        nc.vector.memset(u_sq[:, 1, 1:2], 1.0)

        bq8 = small.tile([128, NCHUNK], F32)
        nc.vector.tensor_scalar_mul(bq8[:], bq_sb[:], 0.125)

        if not mask_trivial:
            mrow = small.tile([1, N], F32)
            nc.sync.dma_start(mrow[:], mneg_d[:])
            mfull = small.tile([128, N], F32)
            nc.vector.tensor_copy(mfull[:], mrow[0:1, :].to_broadcast([128, N]))

        # ---------------- activation / k,v-slice weight loads (gpsimd) -----
        proj = ctx.enter_context(tc.tile_pool(name="proj", bufs=1))
        kinT_sb = proj.tile([128, NCHUNK, N // 2], F32)
        nc.gpsimd.dma_start(kinT_sb[:], kinT_d[:])
        sTb_sb = proj.tile([128, NCHUNK, ROWS // 2], F32)
        nc.gpsimd.dma_start(sTb_sb[:], sT_d[:])
        w_sbs = {}
        if USE_CC:
            for wname, wd, ncol in [("wk", wk_d, 128), ("wv", wv_d, 128)]:
                t = proj.tile([128, NCHUNK, ncol], F32, name=f"w_{wname}")
                nc.gpsimd.dma_start(t[:], wd[:])
                w_sbs[wname] = t
        else:
            for wname, wd in [("wk", wk_d), ("wv", wv_d)]:
                t = proj.tile([128, NCHUNK, CS // 2], F32, name=f"w_{wname}")
                nc.gpsimd.dma_start(t[:], wd[:])
                w_sbs[wname] = t
        for wname, wd in [("wq", wq_d), ("wg", wg_d), ("wo", wo_d)]:
            w_sbs[wname] = proj.tile([128, NCHUNK, CS // 2], F32,
                                     name=f"w_{wname}")

        def wbf(wname, cc, c0, ncol):
            # bf16 view of weight cols [c0, c0+ncol) in chunk cc
            return w_sbs[wname][:, cc, c0 // 2:(c0 + ncol) // 2].bitcast(BF16)

        kT_sb = proj.tile([128, NCHUNK, N], BF16)      # [d, j] full after AG
        v_sb = proj.tile([128, 4, CS], BF16)           # [j in chunk, jc, h*64+d]

        # ---- k/v slice projections (this core's 4 heads), before phase 1
        with ExitStack() as kvctx:
            kvps = kvctx.enter_context(tc.tile_pool(name="kvps", bufs=2,
                                                    space="PSUM"))
            if USE_CC:
                kpart = proj.tile([128, 2, N], BF16)
                for dc in range(2):
                    ps = kvps.tile([128, N], F32, tag="kv")
                    for cc in range(NCHUNK):
                        nc.tensor.matmul(ps[:], wbf("wk", cc, 128 * dc, 128),
                                         kinT_sb[:, cc, :].bitcast(BF16),
                                         start=(cc == 0), stop=(cc == NCHUNK - 1))
                    nc.vector.tensor_copy(kpart[:, dc, :], ps[:])
                vpart = proj.tile([128, 4, 256], BF16)
                for jc in range(4):
                    ps = kvps.tile([128, 256], F32, tag="kv")
                    for cc in range(NCHUNK):
                        nc.tensor.matmul(
                            ps[:],
                            kinT_sb[:, cc, 64 * jc:64 * jc + 64].bitcast(BF16),
                            wbf("wv", cc, 0, 256),
                            start=(cc == 0), stop=(cc == NCHUNK - 1))
                    nc.vector.tensor_copy(vpart[:, jc, :], ps[:])
                # staging writes ride gpsimd: the sync/scalar queues stay
                # pure z-prefetch from t=0 (a blocked write at their head
                # would stall every zin load queued behind it)
                nc.gpsimd.dma_start(
                    kv_in[:, 0:1024],
                    kpart.rearrange("p dc j -> p (dc j)")[:])
                nc.gpsimd.dma_start(
                    kv_in[:, 1024:2048],
                    vpart.rearrange("p jc x -> p (jc x)")[:])
            else:
                for dc in range(NCHUNK):
                    ps = kvps.tile([128, N], F32, tag="kv")
                    for cc in range(NCHUNK):
                        nc.tensor.matmul(ps[:], wbf("wk", cc, 128 * dc, 128),
                                         kinT_sb[:, cc, :].bitcast(BF16),
                                         start=(cc == 0), stop=(cc == NCHUNK - 1))
                    nc.vector.tensor_copy(kT_sb[:, dc, :], ps[:])
                for jc in range(4):
                    for q2 in range(2):
                        ps = kvps.tile([128, 512], F32, tag="kv")
                        for cc in range(NCHUNK):
                            nc.tensor.matmul(
                                ps[:],
                                kinT_sb[:, cc, 64 * jc:64 * jc + 64].bitcast(BF16),
                                wbf("wv", cc, 512 * q2, 512),
                                start=(cc == 0), stop=(cc == NCHUNK - 1))
                        nc.vector.tensor_copy(
                            v_sb[:, jc, 512 * q2:512 * q2 + 512], ps[:])

        # ---------------- phase 1: z -> centered zu / musum / sumsq --------
        # q/g projections are interleaved into the z loop (the PE's z-DMA
        # famine windows do useful work); their inputs are loaded well
        # before the octet that issues them.
        qT_sb = proj.tile([128, NCHUNK, ROWS], BF16)   # (q + bq)/8, [d, i]
        g_sb = proj.tile([128, CS], BF16)              # sigmoid(s @ wg), [i, c]

        QR = 4   # query rows per (group, octet)
        with ExitStack() as zctx:
            ztp = zctx.enter_context(tc.tile_pool(name="ztp", bufs=14))
            z2p = zctx.enter_context(tc.tile_pool(name="z2p", bufs=5))
            zup = zctx.enter_context(tc.tile_pool(name="zup", bufs=3))
            ssp = zctx.enter_context(tc.tile_pool(name="ssp", bufs=3))
            zps = zctx.enter_context(tc.tile_pool(name="zps", bufs=2, space="PSUM"))
            ssps = zctx.enter_context(tc.tile_pool(name="ssps", bufs=4,
                                                   space="PSUM"))
            prps = zctx.enter_context(tc.tile_pool(name="prps", bufs=2,
                                                   space="PSUM"))

            def q_chunk(dc):
                ps = prps.tile([128, ROWS], F32, tag="q")
                for cc in range(NCHUNK):
                    nc.tensor.matmul(ps[:], wbf("wq", cc, 128 * dc, 128),
                                     sTb_sb[:, cc, :].bitcast(BF16),
                                     start=(cc == 0), stop=(cc == NCHUNK - 1))
                nc.vector.tensor_scalar(qT_sb[:, dc, :], ps[:], 0.125,
                                        bq8[:, dc:dc + 1],
                                        op0=ALU.mult, op1=ALU.add)

            def g_chunk(q2):
                ps = prps.tile([128, 512], F32, tag="q")
                for cc in range(NCHUNK):
                    nc.tensor.matmul(ps[:], sTb_sb[:, cc, :].bitcast(BF16),
                                     wbf("wg", cc, 512 * q2, 512),
                                     start=(cc == 0), stop=(cc == NCHUNK - 1))
                nc.scalar.activation(g_sb[:, 512 * q2:512 * q2 + 512], ps[:],
                                     AF.Sigmoid)

            pend_writes = []

            # sumsq plane viewed [1, g, row-in-group, j] for batched writes
            ss_v = zu_d[17:18, :, :].rearrange("o (g r) j -> o g r j", g=4)

            def _flush_writes(o, zu_sb, ss_sb):
                for g in range(4):
                    r0 = 32 * g + QR * o
                    nc.sync.dma_start(zu_d[0:17, r0:r0 + QR, :],
                                      zu_sb[32 * g:32 * g + 17, :, :])
                # one write per psum-row-plane p: rows 32g+4o+2*kkp+p for all
                # (g, kkp) at once
                for p in range(2):
                    nc.sync.dma_start(
                        ss_v[:, :, 4 * o + p:4 * o + p + 3:2, :],
                        ss_sb[p:p + 1, :, :, :])

            for o in range(32 // QR):
                # big weight loads + collective ride the gpsimd queue,
                # spread through phase 1 so z DMA keeps its bandwidth
                if o == 0:
                    nc.gpsimd.dma_start(w_sbs["wq"][:], wq_d[:])
                elif o == 1 and USE_CC:
                    nc.gpsimd.collective_compute(
                        "AllGather", ALU.bypass,
                        replica_groups=[[0, 1, 2, 3], [4, 5, 6, 7]],
                        ins=[kv_in[:].opt()],
                        outs=[kv_out[:].opt()])
                elif o == 2:
                    nc.gpsimd.dma_start(w_sbs["wg"][:], wg_d[:])
                elif o == 4 and USE_CC:
                    # by now the collective is long done; these won't block
                    # the gpsimd queue on its semaphore
                    for r in range(4):
                        nc.gpsimd.dma_start(
                            kT_sb[:, 2 * r:2 * r + 2, :],
                            kv_out[r, :, 0:1024].rearrange(
                                "p (dc j) -> p dc j", dc=2))
                        nc.gpsimd.dma_start(
                            v_sb[:, :, 256 * r:256 * r + 256],
                            kv_out[r, :, 1024:2048].rearrange(
                                "p (jc x) -> p jc x", jc=4))
                elif o == 5:
                    nc.gpsimd.dma_start(w_sbs["wo"][:], wo_d[:])

                if 3 <= o <= 6:
                    q_chunk(2 * (o - 3))
                    q_chunk(2 * (o - 3) + 1)
                if o == 6:
                    g_chunk(0)
                elif o == 7:
                    g_chunk(1)

                zins = []
                for g in range(4):
                    r0 = 32 * g + QR * o
                    zin = ztp.tile([CZ, QR, N // 2], F32, tag="zin")
                    # all phase-1 DMA rides sync: the sync engine runs no
                    # compute, so a DMA issue stalled by the collective's
                    # SDMA burst can't take compute ops hostage (in-order
                    # engine queues)
                    nc.sync.dma_start(zin[:], zh_d[:, r0:r0 + QR, :])
                    z2 = z2p.tile([CZ, QR, N], F8E4, tag="z2")
                    if g < 2:
                        nc.scalar.activation(z2[:], zin[:].bitcast(BF16),
                                             AF.Square)
                    else:
                        nc.vector.tensor_tensor(z2[:], zin[:].bitcast(BF16),
                                                zin[:].bitcast(BF16), ALU.mult)
                    zins.append((zin, z2))
                zu_sb = zup.tile([128, QR, N], BF16)
                # sumsq staging on partitions 0-1: [row-in-pair, g, pair, j]
                ss_sb = ssp.tile([2, 4, QR // 2, N], BF16)
                for kk in range(QR):
                    ps_zu = zps.tile([128, N], F32, tag="pzu")
                    for g in range(4):
                        zin, z2 = zins[g]
                        hi1 = zin[:, kk, :].bitcast(BF16)
                        tp = (0, 32 * g)
                        nc.tensor.matmul(ps_zu[32 * g:32 * g + 32, :],
                                         u_bf[:], hi1,
                                         start=True, stop=True, tile_position=tp)
                    if kk < 2:
                        nc.scalar.copy(zu_sb[:, kk, :], ps_zu[:])
                    else:
                        nc.vector.tensor_copy(zu_sb[:, kk, :], ps_zu[:])
                for kkp in range(QR // 2):
                    for g in range(4):
                        zin, z2 = zins[g]
                        # fp8 DoubleRow: kk-pair as the two k-tiles; stationary
                        # selects k-tile 0 -> out row 0, k-tile 1 -> out row 1
                        ps_ss = ssps.tile([32, N], F32, tag="pss")
                        nc.tensor.matmul(
                            ps_ss[:], u_sq[:],
                            z2[:, 2 * kkp:2 * kkp + 2, :],
                            start=True, stop=True,
                            perf_mode=mybir.MatmulPerfMode.DoubleRow)
                        if g < 2:
                            nc.scalar.copy(ss_sb[:, g, kkp, :], ps_ss[0:2, :])
                        else:
                            nc.vector.tensor_copy(ss_sb[:, g, kkp, :],
                                                  ps_ss[0:2, :])
                # delay the DRAM write issues by 2 octets: the sync/scalar
                # queues stay pure-prefetch ahead of the PE, so a slow octet
                # can't block the zin loads queued behind its writes
                pend_writes.append((o, zu_sb, ss_sb))
                if o == 7:
                    # all zin loads are issued; drain the write backlog now so
                    # the DRAM writes overlap the last octets' compute instead
                    # of serializing in front of the attention phase
                    for w in pend_writes:
                        _flush_writes(*w)
                    pend_writes = []
                elif o >= 2:
                    _flush_writes(*pend_writes.pop(0))

        # ---------------- phase 3: attention ----------------
        att = ctx.enter_context(tc.tile_pool(name="att", bufs=4))
        apool = ctx.enter_context(tc.tile_pool(name="apool", bufs=1))
        spsum = ctx.enter_context(tc.tile_pool(name="spsum", bufs=2, space="PSUM"))
        tpsum = ctx.enter_context(tc.tile_pool(name="tpsum", bufs=2, space="PSUM"))
        opsum = ctx.enter_context(tc.tile_pool(name="opsum", bufs=2, space="PSUM"))

        musum = apool.tile([128, N], BF16)
        nc.scalar.dma_start(musum[:],
                          zu_d[16:17, :, :].rearrange("o i j -> (o i) j"))
        ssq = apool.tile([128, N], BF16)
        nc.scalar.dma_start(ssq[:],
                          zu_d[17:18, :, :].rearrange("o i j -> (o i) j"))
        m2 = apool.tile([128, N], F32)
        nc.vector.tensor_tensor(m2[:], musum[:], musum[:], ALU.mult)
        wvar = apool.tile([128, N], F32)   # 128 * var
        nc.vector.scalar_tensor_tensor(wvar[:], m2[:], -1.0 / CZ, ssq[:],
                                       op0=ALU.mult, op1=ALU.add)
        eps_b = apool.tile([128, 1], F32)
        nc.vector.memset(eps_b[:], EPS)
        sdev = apool.tile([128, N], F32)   # sqrt(var + eps)
        nc.scalar.activation(sdev[:], wvar[:], AF.Sqrt, bias=eps_b[:, 0:1],
                             scale=1.0 / CZ)
        rsig = apool.tile([128, N], F32)
        nc.vector.reciprocal(rsig[:], sdev[:])

        o_all = apool.tile([128, H, D], F32)
        sums = apool.tile([128, H], F32)

        for h in range(H):
            bh = att.tile([128, N], BF16, tag="bh")
            nc.scalar.dma_start(
                bh[:], zu_d[h:h + 1, :, :].rearrange("o i j -> (o i) j"))
            sc_ps = spsum.tile([128, N], F32, tag="sc")
            p0 = 64 * (h % 2)
            nc.tensor.matmul(sc_ps[:],
                             qT_sb[p0:p0 + 64, h // 2, :],
                             kT_sb[p0:p0 + 64, h // 2, :],
                             start=True, stop=True)
            t2 = att.tile([128, N], F32, tag="t2")
            nc.gpsimd.tensor_tensor(t2[:], bh[:], rsig[:], ALU.mult)
            if not mask_trivial:
                nc.vector.tensor_tensor(t2[:], t2[:], mfull[:], ALU.add)
            s_sb = att.tile([128, N], F32, tag="s")
            nc.vector.scalar_tensor_tensor(s_sb[:], t2[:], t_b[:, h:h + 1],
                                           sc_ps[:], op0=ALU.add, op1=ALU.add)
            p_sb = att.tile([128, N], BF16, tag="p")
            nc.scalar.activation(p_sb[:], s_sb[:], AF.Exp,
                                 accum_out=sums[:, h:h + 1])
            pt_ps = tpsum.tile([128, N], BF16, tag="pt")
            for jc in range(4):
                nc.tensor.transpose(pt_ps[:, 128 * jc:128 * jc + 128],
                                    p_sb[:, 128 * jc:128 * jc + 128],
                                    ident_bf[:])
            pt_sb = att.tile([128, N], BF16, tag="ptsb")
            nc.vector.tensor_copy(pt_sb[:], pt_ps[:])
            o_ps = opsum.tile([128, D], F32, tag="o")
            for jc in range(4):
                nc.tensor.matmul(o_ps[:], pt_sb[:, 128 * jc:128 * jc + 128],
                                 v_sb[:, jc, D * h:D * h + D],
                                 start=(jc == 0), stop=(jc == 3))
            nc.scalar.copy(o_all[:, h, :], o_ps[:])

        recip = apool.tile([128, H], F32)
        nc.vector.reciprocal(recip[:], sums[:])
        go = apool.tile([128, H, D], F32)
        nc.vector.tensor_tensor(go[:], o_all[:],
                                recip[:, :, None].to_broadcast([128, H, D]),
                                ALU.mult)
        gof = apool.tile([128, CS], BF16)
        nc.vector.tensor_tensor(gof[:], go.rearrange("p h d -> p (h d)")[:],
                                g_sb[:], ALU.mult)

        goT = apool.tile([128, NCHUNK, ROWS], BF16)
        for ccc in range(NCHUNK):
            gt_ps = tpsum.tile([128, 128], BF16, tag="pt")
            nc.tensor.transpose(gt_ps[:], gof[:, 128 * ccc:128 * ccc + 128],
                                ident_bf[:])
            nc.scalar.copy(goT[:, ccc, :], gt_ps[:])

        out_sb = apool.tile([128, CS], F32)
        for q2 in range(2):
            ps = spsum.tile([128, 512], F32, tag="sc")
            for cc in range(NCHUNK):
                nc.tensor.matmul(ps[:], goT[:, cc, :],
                                 wbf("wo", cc, 512 * q2, 512),
                                 start=(cc == 0), stop=(cc == NCHUNK - 1))
            nc.vector.tensor_copy(out_sb[:, 512 * q2:512 * q2 + 512], ps[:])
        nc.scalar.dma_start(out_d[:], out_sb[:])

    nc.compile()
    return nc


def _prepare(s, z, mask, k_in, wq, bq, wk, wv, wg, ln_g, ln_b, wz, wo,
             multiplicity=1, **_ignored):
    import ml_dtypes
    s = np.asarray(s, dtype=np.float32)
    z = np.asarray(z, dtype=np.float32)
    mask = np.asarray(mask, dtype=np.float32)
    k_in = np.asarray(k_in, dtype=np.float32)
    assert int(multiplicity) == 1, "only multiplicity == 1 is supported"
    mask_trivial = bool(np.all(mask == 1.0))

    def bfpack(a):
        # fp32 array -> bf16, bit-packed pairwise into f32 along last axis
        b = np.ascontiguousarray(a).astype(ml_dtypes.bfloat16)
        return b.view(np.float32)

    def wchunk(w):
        # [1024, 1024] -> [128, 8, 1024] so each partition's data is contiguous
        return np.ascontiguousarray(
            np.asarray(w, dtype=np.float32).reshape(NCHUNK, 128, CS)
            .transpose(1, 0, 2))

    wkc, wvc = wchunk(wk), wchunk(wv)
    shared = {
        "wq": bfpack(wchunk(wq)), "wg": bfpack(wchunk(wg)),
        "wo": bfpack(wchunk(wo)),
        "bqt": np.ascontiguousarray(
            np.asarray(bq, dtype=np.float32).reshape(NCHUNK, 128).T),
        "lng": np.ascontiguousarray(
            np.asarray(ln_g, dtype=np.float32).reshape(CZ, 1)),
        "lnb": np.ascontiguousarray(
            np.asarray(ln_b, dtype=np.float32).reshape(CZ, 1)),
        "wz": np.ascontiguousarray(wz, dtype=np.float32),
    }
    if not USE_CC:
        shared["wk"] = bfpack(wkc)
        shared["wv"] = bfpack(wvc)
    in_maps = []
    for core in range(N_CORES):
        b, ib = core // 4, core % 4
        i0 = ib * ROWS
        m = dict(shared)
        m["sT"] = bfpack(
            s[b, i0:i0 + ROWS, :].T.reshape(NCHUNK, 128, ROWS)
            .transpose(1, 0, 2))
        m["kinT"] = bfpack(
            k_in[b].T.reshape(NCHUNK, 128, N).transpose(1, 0, 2))
        m["zh"] = bfpack(z[b, i0:i0 + ROWS].transpose(2, 0, 1))
        if USE_CC:
            m["wk"] = bfpack(wkc[:, :, 256 * ib:256 * ib + 256])
            m["wv"] = bfpack(wvc[:, :, 256 * ib:256 * ib + 256])
        if not mask_trivial:
            m["mneg"] = np.ascontiguousarray(
                ((1.0 - mask[b]) * -1e6).reshape(1, N))
        in_maps.append(m)
    return mask_trivial, in_maps


def _run(in_maps, mask_trivial, **kwargs):
    if mask_trivial not in _CACHE:
        _CACHE[mask_trivial] = _build_program(mask_trivial)
    nc = _CACHE[mask_trivial]
    res = run_bass_kernel_spmd(nc, in_maps, core_ids=list(range(N_CORES)),
                               **kwargs)
    out = np.empty((B, N, CS), dtype=np.float32)
    for core in range(N_CORES):
        b, ib = core // 4, core % 4
        out[b, ib * ROWS:(ib + 1) * ROWS, :] = res.results[core]["out"]
    return out, res


def kernel(**inputs):
    mask_trivial, in_maps = _prepare(**inputs)
    out, _ = _run(in_maps, mask_trivial)
    return out


def run_profiled(inputs, tmpdir=None):
    mask_trivial, in_maps = _prepare(**inputs)
    out, res = _run(in_maps, mask_trivial, trace=True, tmpdir=tmpdir)
    return out, res


# revision 41
# speedup vs baseline: 1.0542x; 1.0542x over previous
"""AttentionPairBias kernel for 8 Trainium2 NeuronCores (v3, bf16 + cc).

Sharding: data-parallel over (batch, query-row-block). Core c handles batch
b = c // 4 and query rows i in [(c % 4) * 128, (c % 4 + 1) * 128).
Each core computes the full 16-head attention for its 128 query rows.

v3 changes vs v2 (298us):
  - k/v projections are column-sharded over the 4 cores of a batch (each
    core computes 4 heads' worth: kT dc-chunks {2*ib, 2*ib+1}, v head-cols
    [256*ib, 256*ib+256)), then AllGather'd within the 4-core replica group
    via DRAM bounce buffers on the gpsimd ring. Removes the 4x-redundant
    k/v compute (48k PE cycles) and 6MB of weight DMA per core.
  - k/v slice matmuls issue before the z loop; the collective overlaps the
    long z phase; read-back + remaining weight loads ride the gpsimd queue.
  - wq/wg/wo loads are interleaved into the z loop so they don't hog HBM
    bandwidth at t=0 (phase-1 z streaming starts immediately).
  - deeper zin prefetch (8 bufs).

v2 changes vs v1 (427us):
  - z loaded as a single bf16 plane; mean-centering folded into the
    stationary (u' = ln_g*wz - su/128) so phase 1 is 2 matmul passes per
    row; all weights bf16; zu roundtrip bf16 (plane 16 = musum, 17 =
    sumsq); attention in bf16 without softmax max-subtract; DMA spread
    over sync/scalar/gpsimd rings.
"""

import sys

sys.path.insert(0, "/opt/trn_rl_repo")

from contextlib import ExitStack

import numpy as np

import concourse.bacc as bacc
import concourse.bass as bass
import concourse.mybir as mybir
import concourse.tile as tile
from concourse.bass_utils import run_bass_kernel_spmd
from concourse.masks import make_identity

F32 = mybir.dt.float32
BF16 = mybir.dt.bfloat16
F8E4 = mybir.dt.float8e4
AF = mybir.ActivationFunctionType
ALU = mybir.AluOpType

B, N, CS, CZ, H, D = 2, 512, 1024, 128, 16, 64
ROWS = 128          # query rows per core
NCHUNK = CS // 128  # 8 contraction chunks of 128
N_CORES = 8
EPS = 1e-5
USE_CC = True       # AllGather k/v across the 4 cores of a batch

_CACHE = {}


def _build_program(mask_trivial: bool):
    nc = bacc.Bacc("TRN2", target_bir_lowering=False, debug=False,
                   num_devices=N_CORES)

    def din(name, shape):
        return nc.dram_tensor(name, shape, F32, kind="ExternalInput").ap()

    # bf16 tensors bit-packed into f32-typed dram tensors (axon PJRT path
    # prefers f32 jit parameters); bitcast to BF16 on-chip.
    sT_d = din("sT", (128, NCHUNK, ROWS // 2))
    kinT_d = din("kinT", (128, NCHUNK, N // 2))
    zh_d = din("zh", (CZ, ROWS, N // 2))
    wq_d = din("wq", (128, NCHUNK, CS // 2))
    if USE_CC:
        wk_d = din("wk", (128, NCHUNK, 128))   # 256-col slice, bf16-packed
        wv_d = din("wv", (128, NCHUNK, 128))
    else:
        wk_d = din("wk", (128, NCHUNK, CS // 2))
        wv_d = din("wv", (128, NCHUNK, CS // 2))
    wg_d = din("wg", (128, NCHUNK, CS // 2))
    wo_d = din("wo", (128, NCHUNK, CS // 2))
    bq_d = din("bqt", (128, NCHUNK))
    lng_d = din("lng", (CZ, 1))
    lnb_d = din("lnb", (CZ, 1))
    wz_d = din("wz", (CZ, H))
    if not mask_trivial:
        mneg_d = din("mneg", (1, N))
    out_d = nc.dram_tensor("out", (ROWS, CS), F32, kind="ExternalOutput").ap()

    with tile.TileContext(nc) as tc, ExitStack() as ctx:
        dram = ctx.enter_context(tc.tile_pool(name="dram", bufs=1, space="DRAM"))
        # planes 0..15: centered zu per head; 16: musum; 17: sumsq
        zu_d = dram.tile([18, ROWS, N], BF16)
        if USE_CC:
            kv_in = dram.tile([128, 2048], BF16, name="kv_in")
            kv_out = dram.tile([4, 128, 2048], BF16, name="kv_out")

        const = ctx.enter_context(tc.tile_pool(name="const", bufs=1))
        small = ctx.enter_context(tc.tile_pool(name="small", bufs=1))

        ident_bf = const.tile([128, 128], BF16)
        make_identity(nc, ident_bf[:])
        ones = const.tile([128, 128], F32)
        nc.vector.memset(ones[:], 1.0)

        wz_sb = small.tile([CZ, H], F32)
        nc.sync.dma_start(wz_sb[:], wz_d[:])
        lng_sb = small.tile([CZ, 1], F32)
        nc.sync.dma_start(lng_sb[:], lng_d[:])
        lnb_sb = small.tile([CZ, 1], F32)
        nc.sync.dma_start(lnb_sb[:], lnb_d[:])
        bq_sb = small.tile([128, NCHUNK], F32)
        nc.sync.dma_start(bq_sb[:], bq_d[:])

        u_f = small.tile([CZ, H], F32)
        nc.vector.tensor_tensor(u_f[:], wz_sb[:],
                                lng_sb[:, 0:1].to_broadcast([CZ, H]), ALU.mult)
        bwz = small.tile([CZ, H], F32)
        nc.vector.tensor_tensor(bwz[:], wz_sb[:],
                                lnb_sb[:, 0:1].to_broadcast([CZ, H]), ALU.mult)

        msu_b = small.tile([128, H], F32)   # -su[h]/128 replicated on partitions
        t_b = small.tile([128, H], F32)
        with ExitStack() as pctx:
            ppre = pctx.enter_context(tc.tile_pool(name="ppre", bufs=1,
                                                   space="PSUM"))
            su_ps = ppre.tile([128, H], F32, tag="pre")
            nc.tensor.matmul(su_ps[:], ones[:], u_f[:], start=True, stop=True)
            nc.vector.tensor_scalar_mul(msu_b[:], su_ps[:], -1.0 / CZ)
            t_ps = ppre.tile([128, H], F32, tag="pre")
            nc.tensor.matmul(t_ps[:], ones[:], bwz[:], start=True, stop=True)
            nc.vector.tensor_copy(t_b[:], t_ps[:])
        # u' = u - su/128 (mean-centering folded into the stationary), bf16,
        # packed [u'(16) | ones(1) | zeros..] in a 32-wide stationary.
        uc_f = small.tile([CZ, H], F32)
        nc.vector.tensor_tensor(uc_f[:], u_f[:], msu_b[0:CZ, :], ALU.add)
        u_bf = const.tile([CZ, 32], BF16)
        nc.vector.memset(u_bf[:], 0.0)
        nc.vector.tensor_copy(u_bf[:, 0:H], uc_f[:])
        nc.vector.memset(u_bf[:, H:H + 1], 1.0)
        # sq-pass stationary for fp8 DoubleRow: k-tile 0 selects output row 0,
        # k-tile 1 selects row 1, so one 256-col stream yields the per-column
        # sums of both j-halves of z^2 at 0.5 cycles/col
        u_sq = const.tile([CZ, 2, 32], F8E4)
        nc.vector.memset(u_sq[:], 0.0)
        nc.vector.memset(u_sq[:, 0, 0:1], 1.0)
        nc.vector.memset(u_sq[:, 1, 1:2], 1.0)

        bq8 = small.tile([128, NCHUNK], F32)
        nc.vector.tensor_scalar_mul(bq8[:], bq_sb[:], 0.125)

        if not mask_trivial:
            mrow = small.tile([1, N], F32)
            nc.sync.dma_start(mrow[:], mneg_d[:])
            mfull = small.tile([128, N], F32)
            nc.vector.tensor_copy(mfull[:], mrow[0:1, :].to_broadcast([128, N]))

        # ---------------- activation / k,v-slice weight loads (gpsimd) -----
        proj = ctx.enter_context(tc.tile_pool(name="proj", bufs=1))
        kinT_sb = proj.tile([128, NCHUNK, N // 2], F32)
        nc.gpsimd.dma_start(kinT_sb[:], kinT_d[:])
        sTb_sb = proj.tile([128, NCHUNK, ROWS // 2], F32)
        nc.gpsimd.dma_start(sTb_sb[:], sT_d[:])
        w_sbs = {}
        if USE_CC:
            for wname, wd, ncol in [("wk", wk_d, 128), ("wv", wv_d, 128)]:
                t = proj.tile([128, NCHUNK, ncol], F32, name=f"w_{wname}")
                nc.gpsimd.dma_start(t[:], wd[:])
                w_sbs[wname] = t
        else:
            for wname, wd in [("wk", wk_d), ("wv", wv_d)]:
                t = proj.tile([128, NCHUNK, CS // 2], F32, name=f"w_{wname}")
                nc.gpsimd.dma_start(t[:], wd[:])
                w_sbs[wname] = t
        for wname, wd in [("wq", wq_d), ("wg", wg_d), ("wo", wo_d)]:
            w_sbs[wname] = proj.tile([128, NCHUNK, CS // 2], F32,
                                     name=f"w_{wname}")

        def wbf(wname, cc, c0, ncol):
            # bf16 view of weight cols [c0, c0+ncol) in chunk cc
            return w_sbs[wname][:, cc, c0 // 2:(c0 + ncol) // 2].bitcast(BF16)

        kT_sb = proj.tile([128, NCHUNK, N], BF16)      # [d, j] full after AG
        v_sb = proj.tile([128, 4, CS], BF16)           # [j in chunk, jc, h*64+d]

        # ---- k/v slice projections (this core's 4 heads), before phase 1
        with ExitStack() as kvctx:
            kvps = kvctx.enter_context(tc.tile_pool(name="kvps", bufs=2,
                                                    space="PSUM"))
            if USE_CC:
                kpart = proj.tile([128, 2, N], BF16)
                for dc in range(2):
                    ps = kvps.tile([128, N], F32, tag="kv")
                    for cc in range(NCHUNK):
                        nc.tensor.matmul(ps[:], wbf("wk", cc, 128 * dc, 128),
                                         kinT_sb[:, cc, :].bitcast(BF16),
                                         start=(cc == 0), stop=(cc == NCHUNK - 1))
                    nc.vector.tensor_copy(kpart[:, dc, :], ps[:])
                vpart = proj.tile([128, 4, 256], BF16)
                for jc in range(4):
                    ps = kvps.tile([128, 256], F32, tag="kv")
                    for cc in range(NCHUNK):
                        nc.tensor.matmul(
                            ps[:],
                            kinT_sb[:, cc, 64 * jc:64 * jc + 64].bitcast(BF16),
                            wbf("wv", cc, 0, 256),
                            start=(cc == 0), stop=(cc == NCHUNK - 1))
                    nc.vector.tensor_copy(vpart[:, jc, :], ps[:])
                # staging writes ride gpsimd: the sync/scalar queues stay
                # pure z-prefetch from t=0 (a blocked write at their head
                # would stall every zin load queued behind it)
                nc.gpsimd.dma_start(
                    kv_in[:, 0:1024],
                    kpart.rearrange("p dc j -> p (dc j)")[:])
                nc.gpsimd.dma_start(
                    kv_in[:, 1024:2048],
                    vpart.rearrange("p jc x -> p (jc x)")[:])
            else:
                for dc in range(NCHUNK):
                    ps = kvps.tile([128, N], F32, tag="kv")
                    for cc in range(NCHUNK):
                        nc.tensor.matmul(ps[:], wbf("wk", cc, 128 * dc, 128),
                                         kinT_sb[:, cc, :].bitcast(BF16),
                                         start=(cc == 0), stop=(cc == NCHUNK - 1))
                    nc.vector.tensor_copy(kT_sb[:, dc, :], ps[:])
                for jc in range(4):
                    for q2 in range(2):
                        ps = kvps.tile([128, 512], F32, tag="kv")
                        for cc in range(NCHUNK):
                            nc.tensor.matmul(
                                ps[:],
                                kinT_sb[:, cc, 64 * jc:64 * jc + 64].bitcast(BF16),
                                wbf("wv", cc, 512 * q2, 512),
                                start=(cc == 0), stop=(cc == NCHUNK - 1))
                        nc.vector.tensor_copy(
                            v_sb[:, jc, 512 * q2:512 * q2 + 512], ps[:])

        # ---------------- phase 1: z -> centered zu / musum / sumsq --------
        # q/g projections are interleaved into the z loop (the PE's z-DMA
        # famine windows do useful work); their inputs are loaded well
        # before the octet that issues them.
        qT_sb = proj.tile([128, NCHUNK, ROWS], BF16)   # (q + bq)/8, [d, i]
        g_sb = proj.tile([128, CS], BF16)              # sigmoid(s @ wg), [i, c]

        QR = 4   # query rows per (group, octet)
        with ExitStack() as zctx:
            ztp = zctx.enter_context(tc.tile_pool(name="ztp", bufs=14))
            z2p = zctx.enter_context(tc.tile_pool(name="z2p", bufs=5))
            zup = zctx.enter_context(tc.tile_pool(name="zup", bufs=3))
            ssp = zctx.enter_context(tc.tile_pool(name="ssp", bufs=3))
            zps = zctx.enter_context(tc.tile_pool(name="zps", bufs=2, space="PSUM"))
            ssps = zctx.enter_context(tc.tile_pool(name="ssps", bufs=4,
                                                   space="PSUM"))
            prps = zctx.enter_context(tc.tile_pool(name="prps", bufs=2,
                                                   space="PSUM"))

            def q_chunk(dc):
                ps = prps.tile([128, ROWS], F32, tag="q")
                for cc in range(NCHUNK):
                    nc.tensor.matmul(ps[:], wbf("wq", cc, 128 * dc, 128),
                                     sTb_sb[:, cc, :].bitcast(BF16),
                                     start=(cc == 0), stop=(cc == NCHUNK - 1))
                nc.vector.tensor_scalar(qT_sb[:, dc, :], ps[:], 0.125,
                                        bq8[:, dc:dc + 1],
                                        op0=ALU.mult, op1=ALU.add)

            def g_chunk(q2):
                ps = prps.tile([128, 512], F32, tag="q")
                for cc in range(NCHUNK):
                    nc.tensor.matmul(ps[:], sTb_sb[:, cc, :].bitcast(BF16),
                                     wbf("wg", cc, 512 * q2, 512),
                                     start=(cc == 0), stop=(cc == NCHUNK - 1))
                nc.scalar.activation(g_sb[:, 512 * q2:512 * q2 + 512], ps[:],
                                     AF.Sigmoid)

            pend_writes = []

            # sumsq plane viewed [1, g, row-in-group, j] for batched writes
            ss_v = zu_d[17:18, :, :].rearrange("o (g r) j -> o g r j", g=4)

            def _flush_writes(o, zu_sb, ss_sb):
                for g in range(4):
                    r0 = 32 * g + QR * o
                    nc.sync.dma_start(zu_d[0:17, r0:r0 + QR, :],
                                      zu_sb[32 * g:32 * g + 17, :, :])
                # one write per psum-row-plane p: rows 32g+4o+2*kkp+p for all
                # (g, kkp) at once
                for p in range(2):
                    nc.sync.dma_start(
                        ss_v[:, :, 4 * o + p:4 * o + p + 3:2, :],
                        ss_sb[p:p + 1, :, :, :])

            for o in range(32 // QR):
                # big weight loads + collective ride the gpsimd queue,
                # spread through phase 1 so z DMA keeps its bandwidth
                if o == 0:
                    nc.gpsimd.dma_start(w_sbs["wq"][:], wq_d[:])
                elif o == 1 and USE_CC:
                    nc.gpsimd.collective_compute(
                        "AllGather", ALU.bypass,
                        replica_groups=[[0, 1, 2, 3], [4, 5, 6, 7]],
                        ins=[kv_in[:].opt()],
                        outs=[kv_out[:].opt()])
                elif o == 2:
                    nc.gpsimd.dma_start(w_sbs["wg"][:], wg_d[:])
                elif o == 4 and USE_CC:
                    # by now the collective is long done; these won't block
                    # the gpsimd queue on its semaphore
                    for r in range(4):
                        nc.gpsimd.dma_start(
                            kT_sb[:, 2 * r:2 * r + 2, :],
                            kv_out[r, :, 0:1024].rearrange(
                                "p (dc j) -> p dc j", dc=2))
                        nc.gpsimd.dma_start(
                            v_sb[:, :, 256 * r:256 * r + 256],
                            kv_out[r, :, 1024:2048].rearrange(
                                "p (jc x) -> p jc x", jc=4))
                elif o == 5:
                    nc.gpsimd.dma_start(w_sbs["wo"][:], wo_d[:])

                if 3 <= o <= 6:
                    q_chunk(2 * (o - 3))
                    q_chunk(2 * (o - 3) + 1)
                if o == 6:
                    g_chunk(0)
                elif o == 7:
                    g_chunk(1)

                zins = []
                for g in range(4):
                    r0 = 32 * g + QR * o
                    zin = ztp.tile([CZ, QR, N // 2], F32, tag="zin")
                    # all phase-1 DMA rides sync: the sync engine runs no
                    # compute, so a DMA issue stalled by the collective's
                    # SDMA burst can't take compute ops hostage (in-order
                    # engine queues)
                    nc.sync.dma_start(zin[:], zh_d[:, r0:r0 + QR, :])
                    z2 = z2p.tile([CZ, QR, N], F8E4, tag="z2")
                    if g < 2:
                        nc.scalar.activation(z2[:], zin[:].bitcast(BF16),
                                             AF.Square)
                    else:
                        nc.vector.tensor_tensor(z2[:], zin[:].bitcast(BF16),
                                                zin[:].bitcast(BF16), ALU.mult)
                    zins.append((zin, z2))
                zu_sb = zup.tile([128, QR, N], BF16)
                # sumsq staging on partitions 0-1: [row-in-pair, g, pair, j]
                ss_sb = ssp.tile([2, 4, QR // 2, N], BF16)
                for kk in range(QR):
                    ps_zu = zps.tile([128, N], F32, tag="pzu")
                    for g in range(4):
                        zin, z2 = zins[g]
                        hi1 = zin[:, kk, :].bitcast(BF16)
                        tp = (0, 32 * g)
                        nc.tensor.matmul(ps_zu[32 * g:32 * g + 32, :],
                                         u_bf[:], hi1,
                                         start=True, stop=True, tile_position=tp)
                    if kk < 2:
                        nc.scalar.copy(zu_sb[:, kk, :], ps_zu[:])
                    else:
                        nc.vector.tensor_copy(zu_sb[:, kk, :], ps_zu[:])
                for kkp in range(QR // 2):
                    for g in range(4):
                        zin, z2 = zins[g]
                        # fp8 DoubleRow: kk-pair as the two k-tiles; stationary
                        # selects k-tile 0 -> out row 0, k-tile 1 -> out row 1
                        ps_ss = ssps.tile([32, N], F32, tag="pss")
                        nc.tensor.matmul(
                            ps_ss[:], u_sq[:],
                            z2[:, 2 * kkp:2 * kkp + 2, :],
                            start=True, stop=True,
                            perf_mode=mybir.MatmulPerfMode.DoubleRow)
                        if g < 2:
                            nc.scalar.copy(ss_sb[:, g, kkp, :], ps_ss[0:2, :])
                        else:
                            nc.vector.tensor_copy(ss_sb[:, g, kkp, :],
                                                  ps_ss[0:2, :])
                # delay the DRAM write issues by 2 octets: the sync/scalar
                # queues stay pure-prefetch ahead of the PE, so a slow octet
                # can't block the zin loads queued behind its writes
                pend_writes.append((o, zu_sb, ss_sb))
                if o == 7:
                    # all zin loads are issued; drain the write backlog now so
                    # the DRAM writes overlap the last octets' compute instead
                    # of serializing in front of the attention phase
                    for w in pend_writes:
                        _flush_writes(*w)
                    pend_writes = []
                elif o >= 2:
                    _flush_writes(*pend_writes.pop(0))

        # ---------------- phase 3: attention ----------------
        att = ctx.enter_context(tc.tile_pool(name="att", bufs=4))
        apool = ctx.enter_context(tc.tile_pool(name="apool", bufs=1))
        spsum = ctx.enter_context(tc.tile_pool(name="spsum", bufs=2, space="PSUM"))
        tpsum = ctx.enter_context(tc.tile_pool(name="tpsum", bufs=2, space="PSUM"))
        opsum = ctx.enter_context(tc.tile_pool(name="opsum", bufs=2, space="PSUM"))

        musum = apool.tile([128, N], BF16)
        nc.sync.dma_start(musum[:],
                          zu_d[16:17, :, :].rearrange("o i j -> (o i) j"))
        ssq = apool.tile([128, N], BF16)
        nc.sync.dma_start(ssq[:],
                          zu_d[17:18, :, :].rearrange("o i j -> (o i) j"))
        m2 = apool.tile([128, N], F32)
        nc.vector.tensor_tensor(m2[:], musum[:], musum[:], ALU.mult)
        wvar = apool.tile([128, N], F32)   # 128 * var
        nc.vector.scalar_tensor_tensor(wvar[:], m2[:], -1.0 / CZ, ssq[:],
                                       op0=ALU.mult, op1=ALU.add)
        eps_b = apool.tile([128, 1], F32)
        nc.vector.memset(eps_b[:], EPS)
        sdev = apool.tile([128, N], F32)   # sqrt(var + eps)
        nc.scalar.activation(sdev[:], wvar[:], AF.Sqrt, bias=eps_b[:, 0:1],
                             scale=1.0 / CZ)
        rsig = apool.tile([128, N], F32)
        nc.vector.reciprocal(rsig[:], sdev[:])

        o_all = apool.tile([128, H, D], F32)
        sums = apool.tile([128, H], F32)

        for h in range(H):
            bh = att.tile([128, N], BF16, tag="bh")
            nc.sync.dma_start(
                bh[:], zu_d[h:h + 1, :, :].rearrange("o i j -> (o i) j"))
            sc_ps = spsum.tile([128, N], F32, tag="sc")
            p0 = 64 * (h % 2)
            nc.tensor.matmul(sc_ps[:],
                             qT_sb[p0:p0 + 64, h // 2, :],
                             kT_sb[p0:p0 + 64, h // 2, :],
                             start=True, stop=True)
            t2 = att.tile([128, N], F32, tag="t2")
            nc.gpsimd.tensor_tensor(t2[:], bh[:], rsig[:], ALU.mult)
            if not mask_trivial:
                nc.vector.tensor_tensor(t2[:], t2[:], mfull[:], ALU.add)
            s_sb = att.tile([128, N], F32, tag="s")
            nc.vector.scalar_tensor_tensor(s_sb[:], t2[:], t_b[:, h:h + 1],
                                           sc_ps[:], op0=ALU.add, op1=ALU.add)
            p_sb = att.tile([128, N], BF16, tag="p")
            nc.scalar.activation(p_sb[:], s_sb[:], AF.Exp,
                                 accum_out=sums[:, h:h + 1])
            pt_ps = tpsum.tile([128, N], BF16, tag="pt")
            for jc in range(4):
                nc.tensor.transpose(pt_ps[:, 128 * jc:128 * jc + 128],
                                    p_sb[:, 128 * jc:128 * jc + 128],
                                    ident_bf[:])
            pt_sb = att.tile([128, N], BF16, tag="ptsb")
            nc.vector.tensor_copy(pt_sb[:], pt_ps[:])
            o_ps = opsum.tile([128, D], F32, tag="o")
            for jc in range(4):
                nc.tensor.matmul(o_ps[:], pt_sb[:, 128 * jc:128 * jc + 128],
                                 v_sb[:, jc, D * h:D * h + D],
                                 start=(jc == 0), stop=(jc == 3))
            nc.scalar.copy(o_all[:, h, :], o_ps[:])

        recip = apool.tile([128, H], F32)
        nc.vector.reciprocal(recip[:], sums[:])
        go = apool.tile([128, H, D], F32)
        nc.vector.tensor_tensor(go[:], o_all[:],
                                recip[:, :, None].to_broadcast([128, H, D]),
                                ALU.mult)
        gof = apool.tile([128, CS], BF16)
        nc.vector.tensor_tensor(gof[:], go.rearrange("p h d -> p (h d)")[:],
                                g_sb[:], ALU.mult)

        goT = apool.tile([128, NCHUNK, ROWS], BF16)
        for ccc in range(NCHUNK):
            gt_ps = tpsum.tile([128, 128], BF16, tag="pt")
            nc.tensor.transpose(gt_ps[:], gof[:, 128 * ccc:128 * ccc + 128],
                                ident_bf[:])
            nc.scalar.copy(goT[:, ccc, :], gt_ps[:])

        out_sb = apool.tile([128, CS], F32)
        for q2 in range(2):
            ps = spsum.tile([128, 512], F32, tag="sc")
            for cc in range(NCHUNK):
                nc.tensor.matmul(ps[:], goT[:, cc, :],
                                 wbf("wo", cc, 512 * q2, 512),
                                 start=(cc == 0), stop=(cc == NCHUNK - 1))
            nc.vector.tensor_copy(out_sb[:, 512 * q2:512 * q2 + 512], ps[:])
        nc.sync.dma_start(out_d[:], out_sb[:])

    nc.compile()
    return nc


def _prepare(s, z, mask, k_in, wq, bq, wk, wv, wg, ln_g, ln_b, wz, wo,
             multiplicity=1, **_ignored):
    import ml_dtypes
    s = np.asarray(s, dtype=np.float32)
    z = np.asarray(z, dtype=np.float32)
    mask = np.asarray(mask, dtype=np.float32)
    k_in = np.asarray(k_in, dtype=np.float32)
    assert int(multiplicity) == 1, "only multiplicity == 1 is supported"
    mask_trivial = bool(np.all(mask == 1.0))

    def bfpack(a):
        # fp32 array -> bf16, bit-packed pairwise into f32 along last axis
        b = np.ascontiguousarray(a).astype(ml_dtypes.bfloat16)
        return b.view(np.float32)

    def wchunk(w):
        # [1024, 1024] -> [128, 8, 1024] so each partition's data is contiguous
        return np.ascontiguousarray(
            np.asarray(w, dtype=np.float32).reshape(NCHUNK, 128, CS)
            .transpose(1, 0, 2))

    wkc, wvc = wchunk(wk), wchunk(wv)
    shared = {
        "wq": bfpack(wchunk(wq)), "wg": bfpack(wchunk(wg)),
        "wo": bfpack(wchunk(wo)),
        "bqt": np.ascontiguousarray(
            np.asarray(bq, dtype=np.float32).reshape(NCHUNK, 128).T),
        "lng": np.ascontiguousarray(
            np.asarray(ln_g, dtype=np.float32).reshape(CZ, 1)),
        "lnb": np.ascontiguousarray(
            np.asarray(ln_b, dtype=np.float32).reshape(CZ, 1)),
        "wz": np.ascontiguousarray(wz, dtype=np.float32),
    }
    if not USE_CC:
        shared["wk"] = bfpack(wkc)
        shared["wv"] = bfpack(wvc)
    in_maps = []
    for core in range(N_CORES):
        b, ib = core // 4, core % 4
        i0 = ib * ROWS
        m = dict(shared)
        m["sT"] = bfpack(
            s[b, i0:i0 + ROWS, :].T.reshape(NCHUNK, 128, ROWS)
            .transpose(1, 0, 2))
        m["kinT"] = bfpack(
            k_in[b].T.reshape(NCHUNK, 128, N).transpose(1, 0, 2))
        m["zh"] = bfpack(z[b, i0:i0 + ROWS].transpose(2, 0, 1))
        if USE_CC:
            m["wk"] = bfpack(wkc[:, :, 256 * ib:256 * ib + 256])
            m["wv"] = bfpack(wvc[:, :, 256 * ib:256 * ib + 256])
        if not mask_trivial:
            m["mneg"] = np.ascontiguousarray(
                ((1.0 - mask[b]) * -1e6).reshape(1, N))
        in_maps.append(m)
    return mask_trivial, in_maps


def _run(in_maps, mask_trivial, **kwargs):
    if mask_trivial not in _CACHE:
        _CACHE[mask_trivial] = _build_program(mask_trivial)
    nc = _CACHE[mask_trivial]
    res = run_bass_kernel_spmd(nc, in_maps, core_ids=list(range(N_CORES)),
                               **kwargs)
    out = np.empty((B, N, CS), dtype=np.float32)
    for core in range(N_CORES):
        b, ib = core // 4, core % 4
        out[b, ib * ROWS:(ib + 1) * ROWS, :] = res.results[core]["out"]
    return out, res


def kernel(**inputs):
    mask_trivial, in_maps = _prepare(**inputs)
    out, _ = _run(in_maps, mask_trivial)
    return out


def run_profiled(inputs, tmpdir=None):
    mask_trivial, in_maps = _prepare(**inputs)
    out, res = _run(in_maps, mask_trivial, trace=True, tmpdir=tmpdir)
    return out, res
